# revision 21
# baseline (speedup 1.0000x reference)
"""Trainium2 Bass kernel for nn_DeepQNetIVCML (GNN message passing).

Reference computation per (b, a) pair:
  multi-hop coverage over a sparse binary adjacency (3 steps), weighted
  feature aggregation, mask + mean-normalize, then a small shared MLP.

Sharding: 128 (b, a) pairs split across 8 cores (16 pairs each; every
core sees exactly one b). MLP weights are replicated.

Key kernel ideas (v2 — DMA-roofline oriented):
  - The kernel is DMA-bound; bytes moved per core set the floor. Features
    stream as a SINGLE bf16 copy (no lo-half): per-pair bf16 rounding is
    incoherent across nodes, contributing only ~3e-4 end-to-end rel err.
    W1 and the neighbor half of W2 are bf16 too. The q path (q, W2's q
    half, qb2) stays fp32: its rounding is shared by every (b, a) output
    (coherent), and measured at ~5e-3 if cast — the dominant term.
  - Propagation runs in "path count" space: p_{t+1} = A^T p_t, with
    cover_t = min(prefix_sum, 1); the per-node weight telescopes into a
    linear combination of covers (exact dyadic coefficients, ALPHA^4
    folded into the per-pair scale). A and the seed are binary -> exact
    in fp8.
  - fea is computed with F chunks as the STATIONARY operand and the
    per-node weight column moving, so each matmul emits one fp32 PSUM
    column of the TRANSPOSED [D, pair] activation directly: no wide
    moving streams, no PE transposes, and the relu+mask/den scale is a
    per-pair strided ACT op straight out of PSUM.
  - All per-step elementwise state is batched 4 pairs wide (one quad)
    to amortize DVE fixed costs.
  - All 8 duo input tiles are SBUF-resident simultaneously so the DMA
    stream never flow-controls on compute. W2n is issued LAST on the SP
    ring, after the final feature tile: the post-stream tail is only
    h2 -> relu -> y -> writeback.
"""

import os
import sys

for _p in ("/opt/trn_rl_repo", "/opt/pypackages"):
    if os.path.isdir(_p) and _p not in sys.path:
        sys.path.insert(0, _p)

import ml_dtypes
import numpy as np

import concourse.bacc as bacc
import concourse.mybir as mybir
from concourse.tile import TileContext

B, A, N, D, L = 4, 32, 512, 768, 128
ALPHA = 0.8
STEP_NUM = 3
NCORES = 8
P_PER = (B * A) // NCORES  # pairs per core (16)
NCH = N // 128             # node chunks (4)
DG = D // 128              # feature chunks (6)
NDUO = P_PER // 2          # DMA granule: 2 pairs (8 duos)
NQUAD = P_PER // 4         # compute granule: 4 pairs (4 quads)

BF16 = mybir.dt.bfloat16
F8 = mybir.dt.float8e4
F32 = mybir.dt.float32
BF16_NP = ml_dtypes.bfloat16
F8_NP = ml_dtypes.float8_e4m3

_PROG = None
LAST_RESULT = None


def _build():
    nc = bacc.Bacc("TRN2", target_bir_lowering=False, debug=False,
                   num_devices=NCORES)

    DUO_A = 2 * NCH * N   # A cols per duo
    DUO_F = 2 * NCH * D   # F cols per duo

    a_pre = nc.dram_tensor("a_pre", [128, P_PER * NCH * N], F8,
                           kind="ExternalInput")
    f_pre = nc.dram_tensor("f_pre", [128, P_PER * NCH * D], BF16,
                           kind="ExternalInput")
    s0_pre = nc.dram_tensor("s0_pre", [128, P_PER * NCH], F8,
                            kind="ExternalInput")
    mask_pre = nc.dram_tensor("mask_pre", [1, P_PER], F32,
                              kind="ExternalInput")
    q_pre = nc.dram_tensor("q_pre", [L, D], F32, kind="ExternalInput")
    w1_pre = nc.dram_tensor("w1_pre", [128, DG * D], BF16,
                            kind="ExternalInput")
    w2n_pre = nc.dram_tensor("w2n_pre", [128, DG * D], BF16,
                             kind="ExternalInput")
    w2q_pre = nc.dram_tensor("w2q_pre", [128, DG * D], BF16,
                             kind="ExternalInput")
    w3_pre = nc.dram_tensor("w3_pre", [128, DG], F32, kind="ExternalInput")
    b1_pre = nc.dram_tensor("b1_pre", [1, D], F32, kind="ExternalInput")
    b2_pre = nc.dram_tensor("b2_pre", [1, D], F32, kind="ExternalInput")
    b3_pre = nc.dram_tensor("b3_pre", [1, 1], F32, kind="ExternalInput")
    y_out = nc.dram_tensor("y", [P_PER, 1], F32, kind="ExternalOutput")

    mult = mybir.AluOpType.mult
    add = mybir.AluOpType.add
    relu = mybir.ActivationFunctionType.Relu

    # per-cover weights scaled by ALPHA^-4: exact dyadic rationals
    c_init = 1.0 / ALPHA**3 - 1.0 / ALPHA**2       # 0.390625
    coefs = [1.0 / ALPHA**2 - 1.0 / ALPHA,         # 0.3125
             1.0 / ALPHA - 1.0,                    # 0.25
             1.0]
    a4 = float(np.float32(ALPHA) ** 4)

    with TileContext(nc) as tc:
        with (
            tc.tile_pool(name="const", bufs=1) as cpool,
            tc.tile_pool(name="weights", bufs=1) as wpool,
            tc.tile_pool(name="abuf", bufs=NDUO) as apool,
            tc.tile_pool(name="fbuf", bufs=NDUO) as fpool,
            tc.tile_pool(name="small", bufs=3) as spool,
            tc.tile_pool(name="mlp", bufs=1) as mpool,
        ):
            onesL = cpool.tile([128, 1], F32)
            nc.vector.memset(onesL[:], 1.0 / L)
            ones128 = cpool.tile([128, 1], F32)
            nc.vector.memset(ones128[:], 1.0)
            ones16 = cpool.tile([1, P_PER], F32)
            nc.vector.memset(ones16[:], 1.0)
            ones_row = cpool.tile([1, 128], F32)
            nc.vector.memset(ones_row[:], 1.0)

            # ---- DMA issue. SP ring carries the bulk stream in the exact
            # order compute consumes it; W2n rides the SAME FIFO *after*
            # the last feature tile so the post-stream tail is only the
            # h2 -> y chain. Small/q-path tensors ride the ACT ring.
            s0_sb = cpool.tile([128, P_PER * NCH], F8)
            nc.sync.dma_start(s0_sb[:], s0_pre[:])
            # per quad: both A tiles land BEFORE the F tiles, so the
            # propagation + den/inv/ubf chain (which needs only A) is done
            # by the time the features arrive and fea can fire immediately
            duo = [None] * NDUO
            for qd in range(NQUAD):
                for d in (2 * qd, 2 * qd + 1):
                    A_sb = apool.tile([128, DUO_A], F8, tag="A")
                    nc.sync.dma_start(A_sb[:],
                                      a_pre[:, d * DUO_A:(d + 1) * DUO_A])
                    duo[d] = [A_sb, None]
                for d in (2 * qd, 2 * qd + 1):
                    F_sb = fpool.tile([128, DUO_F], BF16, tag="F")
                    nc.sync.dma_start(F_sb[:],
                                      f_pre[:, d * DUO_F:(d + 1) * DUO_F])
                    duo[d][1] = F_sb
                if qd == 0:
                    mask_sb = cpool.tile([1, P_PER], F32)
                    nc.sync.dma_start(mask_sb[:], mask_pre[:])
            w2n_sb = wpool.tile([128, DG * D], BF16)
            nc.sync.dma_start(w2n_sb[:], w2n_pre[:])

            q_sb = cpool.tile([L, D], F32)
            nc.scalar.dma_start(q_sb[:], q_pre[:])
            w2q_sb = wpool.tile([128, DG * D], BF16)
            nc.scalar.dma_start(w2q_sb[:], w2q_pre[:])
            w1_sb = wpool.tile([128, DG * D], BF16)
            nc.scalar.dma_start(w1_sb[:], w1_pre[:])
            w3_sb = wpool.tile([128, DG], F32)
            nc.scalar.dma_start(w3_sb[:], w3_pre[:])
            b1_sb = cpool.tile([1, D], F32)
            nc.scalar.dma_start(b1_sb[:], b1_pre[:])
            b2_sb = cpool.tile([1, D], F32)
            nc.scalar.dma_start(b2_sb[:], b2_pre[:])
            b3_sb = cpool.tile([1, 1], F32)
            nc.scalar.dma_start(b3_sb[:], b3_pre[:])

            nfT = mpool.tile([128, DG * P_PER], BF16)
            h1T = mpool.tile([128, DG * P_PER], BF16)
            h2T = mpool.tile([128, DG * P_PER], F32)
            qb2_sb = mpool.tile([1, D], F32)

            with (
                tc.tile_pool(name="ppps", bufs=1, space="PSUM") as pp_psum,
                tc.tile_pool(name="denps", bufs=1, space="PSUM") as dn_psum,
                tc.tile_pool(name="feaps", bufs=1, space="PSUM") as ft_psum,
                tc.tile_pool(name="qtps", bufs=1, space="PSUM") as qt_psum,
                tc.tile_pool(name="qrps", bufs=1, space="PSUM") as qr_psum,
            ):
                feaT = ft_psum.tile([128, DG * P_PER], F32, tag="ft")

                def q_block():
                    # q-side of the MLP: q and qmean stay fp32 (their
                    # rounding is coherent across every output); W2q rides
                    # as bf16 and qmean enters the matmul as a bf16 hi+lo
                    # pair, so only the W2q rounding itself survives.
                    # Placed mid-loop so the weight-DMA waits never
                    # head-block the PE FIFO.
                    qm = mpool.tile([128, DG], F32)
                    for g in range(DG):
                        qtp = qt_psum.tile([128, 1], F32, tag="qt")
                        nc.tensor.matmul(qtp[:], q_sb[:, g * 128:(g + 1) * 128],
                                         onesL[:], start=True, stop=True)
                        nc.vector.tensor_copy(qm[:, g:g + 1], qtp[:])
                    qT = mpool.tile([128, 2 * DG], BF16)
                    nc.vector.tensor_copy(qT[:, 0:DG], qm[:])
                    nc.vector.tensor_sub(qT[:, DG:2 * DG], qm[:], qT[:, 0:DG])
                    qrow = qr_psum.tile([1, D], F32, tag="qr")
                    for lo, hi in ((0, 512), (512, D)):
                        for i, (g, col) in enumerate(
                                [(g, c) for g in range(DG)
                                 for c in (g, DG + g)]):
                            nc.tensor.matmul(
                                qrow[:, lo:hi], qT[:, col:col + 1],
                                w2q_sb[:, g * D + lo:g * D + hi],
                                start=(i == 0), stop=(i == 2 * DG - 1))
                    nc.vector.tensor_copy(qb2_sb[:], qrow[:])

                def prop_quad(qd):
                    A_lo = duo[2 * qd][0]
                    A_hi = duo[2 * qd + 1][0]
                    s0c = s0_sb[:, qd * 16:(qd + 1) * 16]
                    pcur = spool.tile([128, 16], F8, tag="pcur")
                    nc.vector.tensor_copy(pcur[:], s0c)
                    pref = spool.tile([128, 16], F32, tag="pref")
                    nc.vector.tensor_copy(pref[:], s0c)
                    wcol = spool.tile([128, 16], F32, tag="wcol")
                    nc.vector.tensor_scalar_mul(wcol[:], pref[:], c_init)
                    ct = spool.tile([128, 16], F32, tag="ct")
                    for t in range(STEP_NUM):
                        ps = pp_psum.tile([128, 16], F32, tag="pp")
                        for pl in range(4):
                            Atile = A_lo if pl < 2 else A_hi
                            poff = (pl % 2) * NCH * N
                            for oc in range(NCH):
                                col = pl * 4 + oc
                                for ic in range(NCH):
                                    nc.tensor.matmul(
                                        ps[:, col:col + 1],
                                        Atile[:, poff + ic * N + oc * 128:
                                              poff + ic * N + oc * 128 + 128],
                                        pcur[:, pl * 4 + ic:pl * 4 + ic + 1],
                                        start=(ic == 0), stop=(ic == NCH - 1))
                        # clamp to {0,1} so the fp8 cast is exact (e4m3
                        # overflows above 448; path counts can exceed)
                        pnext = spool.tile([128, 16], F8, tag="pcur")
                        nc.vector.tensor_scalar_min(pnext[:], ps[:], 1.0)
                        nc.vector.tensor_add(pref[:], pref[:], ps[:])
                        nc.vector.tensor_scalar_min(ct[:], pref[:], 1.0)
                        nc.vector.scalar_tensor_tensor(
                            wcol[:], ct[:], coefs[t], wcol[:],
                            op0=mult, op1=add)
                        pcur = pnext
                    dps = dn_psum.tile([1, 16], F32, tag="dn")
                    nc.tensor.matmul(dps[:], ones128[:], ct[:],
                                     start=True, stop=True)
                    den4 = spool.tile([1, 4], F32, tag="den4")
                    for pl in range(4):
                        nc.vector.tensor_reduce(
                            den4[:, pl:pl + 1], dps[:, pl * 4:(pl + 1) * 4],
                            axis=mybir.AxisListType.X, op=add)
                    # coverage count is an integer >= 1 unless the seed set
                    # is empty (w == 0 there, so any scale gives 0)
                    nc.vector.tensor_scalar_max(den4[:], den4[:], 0.5)
                    rec4 = spool.tile([1, 4], F32, tag="rec4")
                    nc.vector.reciprocal(rec4[:], den4[:])
                    inv4 = spool.tile([1, 4], F32, tag="inv4")
                    nc.vector.scalar_tensor_tensor(
                        inv4[:], rec4[:], a4, mask_sb[:, qd * 4:qd * 4 + 4],
                        op0=mult, op1=mult)
                    # fold the per-pair mask/den/ALPHA^4 scale into the
                    # bf16 weight column; DVE scalar operands are
                    # per-partition, so broadcast inv4 via a rank-1 matmul
                    invb_ps = dn_psum.tile([128, 4], F32, tag="invb")
                    nc.tensor.matmul(invb_ps[:], ones_row[:], inv4[:],
                                     start=True, stop=True)
                    invb = spool.tile([128, 4], F32, tag="invb")
                    nc.vector.tensor_copy(invb[:], invb_ps[:])
                    ubf = spool.tile([128, 16], BF16, tag="ubf")
                    for pl in range(4):
                        nc.vector.tensor_scalar_mul(
                            ubf[:, pl * 4:(pl + 1) * 4],
                            wcol[:, pl * 4:(pl + 1) * 4],
                            invb[:, pl:pl + 1])
                    return dict(ubf=ubf)

                def fea_quad(qd, st):
                    # F chunks stationary, weight column moving: each mm
                    # emits one fp32 column of the transposed activation.
                    # feaT/nfT are PAIR-major so the quad's relu is a single
                    # contiguous op — strided slices here would be tracked
                    # conservatively and serialize mm -> relu across pairs.
                    for pl in range(4):
                        p = qd * 4 + pl
                        Ft = duo[2 * qd + (pl // 2)][1]
                        poff = (pl % 2) * NCH * D
                        for g in range(DG):
                            col = p * DG + g
                            for c in range(NCH):
                                nc.tensor.matmul(
                                    feaT[:, col:col + 1],
                                    Ft[:, poff + c * D + g * 128:
                                       poff + c * D + g * 128 + 128],
                                    st["ubf"][:, pl * 4 + c:pl * 4 + c + 1],
                                    start=(c == 0), stop=(c == NCH - 1))
                    nc.vector.tensor_scalar_max(
                        nfT[:, qd * 4 * DG:(qd + 1) * 4 * DG],
                        feaT[:, qd * 4 * DG:(qd + 1) * 4 * DG], 0.0)

                pending = None
                for qd in range(NQUAD):
                    if pending is not None:
                        fea_quad(qd - 1, pending)
                    pending = prop_quad(qd)
                    if qd == 2:
                        q_block()
                fea_quad(NQUAD - 1, pending)

                # ---- MLP over all 16 pairs; weight-stationary matmuls
                # keep every output in transposed (column) layout; biases
                # enter as rank-1 accumulate matmuls (bias_row^T x ones).
                # nfT is pair-major: view as [128, p, g] and slice per g for
                # the stride-DG moving operands
                nfT_v = nfT[:].rearrange("a (p g) -> a p g", g=DG)
                h1ps = ft_psum.tile([128, DG * P_PER], F32, tag="ft")
                for go in range(DG):
                    sl = slice(go * P_PER, (go + 1) * P_PER)
                    for g in range(DG):
                        nc.tensor.matmul(
                            h1ps[:, sl],
                            w1_sb[:, g * D + go * 128:g * D + go * 128 + 128],
                            nfT_v[:, :, g],
                            start=(g == 0), stop=False)
                    nc.tensor.matmul(
                        h1ps[:, sl], b1_sb[:, go * 128:(go + 1) * 128],
                        ones16[:], start=False, stop=True)
                nc.vector.tensor_scalar_max(h1T[:], h1ps[:], 0.0)
                # dummy matmuls gated on h1T: keep the PE p-state streak
                # alive across the W2n-DMA wait so the h2/y stages dispatch
                # at ramped clock instead of cold
                warm = pp_psum.tile([128, 16], F32, tag="pp")
                for i in range(56):
                    nc.tensor.matmul(warm[:], w1_sb[:, 0:128],
                                     h1T[:, 0:P_PER], start=True, stop=True)

                h2ps = ft_psum.tile([128, DG * P_PER], F32, tag="ft")
                for go in range(DG):
                    sl = slice(go * P_PER, (go + 1) * P_PER)
                    for g in range(DG):
                        nc.tensor.matmul(
                            h2ps[:, sl],
                            w2n_sb[:, g * D + go * 128:g * D + go * 128 + 128],
                            h1T[:, g * P_PER:(g + 1) * P_PER],
                            start=(g == 0), stop=False)
                    nc.tensor.matmul(
                        h2ps[:, sl], qb2_sb[:, go * 128:(go + 1) * 128],
                        ones16[:], start=False, stop=False)
                    nc.tensor.matmul(
                        h2ps[:, sl], b2_sb[:, go * 128:(go + 1) * 128],
                        ones16[:], start=False, stop=True)
                nc.vector.tensor_scalar_max(h2T[:], h2ps[:], 0.0)

                yp = qt_psum.tile([P_PER, 1], F32, tag="yp")
                for g in range(DG):
                    nc.tensor.matmul(yp[:], h2T[:, g * P_PER:(g + 1) * P_PER],
                                     w3_sb[:, g:g + 1],
                                     start=(g == 0), stop=False)
                nc.tensor.matmul(yp[:], ones16[:], b3_sb[:],
                                 start=False, stop=True)
                ysb = mpool.tile([P_PER, 1], F32)
                nc.vector.tensor_copy(ysb[:], yp[:])
                nc.sync.dma_start(y_out[:], ysb[:])

    nc.compile()
    return nc


def get_program():
    global _PROG
    if _PROG is None:
        _PROG = _build()
    return _PROG


def _prep_core(core, query_fea, a_nei, vec_nei, fea_emb, nei_mask,
               W1, b1, W2, b2, W3, b3):
    b = (core * P_PER) // A
    a0 = (core * P_PER) % A
    a_loc = a_nei[b, a0:a0 + P_PER]
    f_loc = fea_emb[b, a0:a0 + P_PER]
    s_loc = vec_nei[b, a0:a0 + P_PER]
    return {
        "a_pre": np.ascontiguousarray(
            a_loc.reshape(P_PER, NCH, 128, N).transpose(2, 0, 1, 3)
            .reshape(128, P_PER * NCH * N)).astype(F8_NP),
        "f_pre": np.ascontiguousarray(
            f_loc.reshape(P_PER, NCH, 128, D).transpose(2, 0, 1, 3)
            .reshape(128, P_PER * NCH * D)).astype(BF16_NP),
        "s0_pre": np.ascontiguousarray(
            s_loc.reshape(P_PER, NCH, 128).transpose(2, 0, 1)
            .reshape(128, P_PER * NCH)).astype(F8_NP),
        "mask_pre": nei_mask[b, a0:a0 + P_PER, 0].reshape(1, P_PER)
        .astype(np.float32),
        "q_pre": query_fea[b].astype(np.float32),
        "w1_pre": np.ascontiguousarray(
            W1.reshape(DG, 128, D).transpose(1, 0, 2).reshape(128, DG * D))
        .astype(BF16_NP),
        "w2n_pre": np.ascontiguousarray(
            W2[:D].reshape(DG, 128, D).transpose(1, 0, 2)
            .reshape(128, DG * D)).astype(BF16_NP),
        "w2q_pre": np.ascontiguousarray(
            W2[D:].reshape(DG, 128, D).transpose(1, 0, 2)
            .reshape(128, DG * D)).astype(BF16_NP),
        "w3_pre": np.ascontiguousarray(
            W3[:, 0].reshape(DG, 128).transpose(1, 0)).astype(np.float32),
        "b1_pre": b1.reshape(1, D).astype(np.float32),
        "b2_pre": b2.reshape(1, D).astype(np.float32),
        "b3_pre": b3.reshape(1, 1).astype(np.float32),
    }


_EXEC = None


def _make_exec():
    """Replicates bass2jax.run_bass_via_pjrt's multi-core path, but caches
    the jitted executable so repeated calls (and timing loops) skip
    recompilation."""
    global _EXEC
    if _EXEC is not None:
        return _EXEC
    import jax
    from jax.experimental.shard_map import shard_map
    from jax.sharding import Mesh, PartitionSpec

    from concourse import mybir as _mybir
    from concourse.bass2jax import (_bass_exec_p, install_neuronx_cc_hook,
                                    partition_id_tensor)

    nc = get_program()
    install_neuronx_cc_hook()
    partition_name = (nc.partition_id_tensor.name
                      if nc.partition_id_tensor else None)
    in_names, out_names, out_avals, zero_outs = [], [], [], []
    for alloc in nc.m.functions[0].allocations:
        if not isinstance(alloc, _mybir.MemoryLocationSet):
            continue
        name = alloc.memorylocations[0].name
        if alloc.kind == "ExternalInput":
            if name != partition_name:
                in_names.append(name)
        elif alloc.kind == "ExternalOutput":
            shape = tuple(alloc.tensor_shape)
            dtype = _mybir.dt.np(alloc.dtype)
            out_names.append(name)
            out_avals.append(jax.core.ShapedArray(shape, dtype))
            zero_outs.append(np.zeros(shape, dtype))
    n_params = len(in_names)
    all_in_names = list(in_names) + list(out_names)
    if partition_name is not None:
        all_in_names.append(partition_name)

    def _body(*args):
        operands = list(args)
        if partition_name is not None:
            operands.append(partition_id_tensor())
        outs = _bass_exec_p.bind(
            *operands,
            out_avals=tuple(out_avals),
            in_names=tuple(all_in_names),
            out_names=tuple(out_names),
            lowering_input_output_aliases=(),
            sim_require_finite=True,
            sim_require_nnan=True,
            nc=nc,
        )
        return tuple(outs)

    devices = jax.devices()[:NCORES]
    mesh = Mesh(np.asarray(devices), ("core",))
    n_outs = len(out_names)
    sharded = jax.jit(
        shard_map(_body, mesh=mesh,
                  in_specs=(PartitionSpec("core"),) * (n_params + n_outs),
                  out_specs=(PartitionSpec("core"),) * n_outs,
                  check_rep=False),
        keep_unused=True,
    )
    _EXEC = (sharded, in_names, out_names, out_avals, zero_outs, mesh)
    return _EXEC


def run_sharded(in_maps, reps=1):
    """Execute on 8 cores; returns (per-core results, [wall_ns per rep])."""
    import time as _time

    import jax

    sharded, in_names, out_names, out_avals, zero_outs, mesh = _make_exec()
    from jax.sharding import NamedSharding, PartitionSpec
    shard = NamedSharding(mesh, PartitionSpec("core"))
    concat_in = [
        jax.device_put(
            np.concatenate([np.asarray(in_maps[c][n])
                            for c in range(NCORES)], axis=0), shard)
        for n in in_names
    ]
    concat_zeros = [
        jax.device_put(
            np.zeros((NCORES * z.shape[0], *z.shape[1:]), z.dtype), shard)
        for z in zero_outs
    ]
    args = concat_in + concat_zeros
    jax.block_until_ready(args)
    out_arrs = None
    times = []
    for _ in range(max(1, reps)):
        t0 = _time.perf_counter()
        out_arrs = sharded(*args)
        jax.block_until_ready(out_arrs)
        times.append((_time.perf_counter() - t0) * 1e9)
    results = [
        {
            name: np.asarray(out_arrs[i]).reshape(
                NCORES, *out_avals[i].shape)[c]
            for i, name in enumerate(out_names)
        }
        for c in range(NCORES)
    ]
    return results, times


def kernel(query_fea, a_nei, vec_nei, fea_emb, nei_mask,
           W1, b1, W2, b2, W3, b3, trace=False, reps=1):
    global LAST_RESULT
    args = [np.asarray(x) for x in (query_fea, a_nei, vec_nei, fea_emb,
                                    nei_mask, W1, b1, W2, b2, W3, b3)]
    in_maps = [_prep_core(c, *args) for c in range(NCORES)]
    results, times = run_sharded(in_maps, reps=reps)
    LAST_RESULT = {"times_ns": times}
    ys = [results[c]["y"].reshape(P_PER) for c in range(NCORES)]
    return np.concatenate(ys).reshape(B, A, 1).astype(np.float32)
